# revision 19
# baseline (speedup 1.0000x reference)
"""Trainium2 Bass kernel for nn_AttentionMask_13048110645633.

Math: for key (4,32,64,64) and query (4,512), with s = key.reshape(B,J) and
q = query, the reference computes elementwise

    ctx[b,j] = sum_k q[b,k]*exp(s[b,j]*q[b,k]) / sum_k exp(s[b,j]*q[b,k])
    out[b,j] = s[b,j] * sigmoid(ctx[b,j])

i.e. out = s * g_b(s) with g_b a smooth scalar gate determined by q[b].
Sharding: data-parallel over B (4 batches x 2 half-slabs = 8 cores), one
(128,512) tile per core.

Device algorithm (per core):
  fit phase  - 128 fit nodes s_n synthesized on-device (iota+affine, no DMA);
               q broadcast to all partitions by a C=2 fp16-pair PE matmul;
               one ACT exp with per-partition scale + accumulate gives
               En=exp(s_n q) and S0; one DVE stt multiply-reduce gives
               0.5*S1; tanh(S1/(2*S0)) on ACT; the sigmoid affine and the
               least-squares fit both fold into a host fit matrix applied by
               two tiny PE matmuls -> even/odd polynomial coefficients of
               g in u = tanh(0.4 s), replicated on all 128 partitions.
  eval phase - u from one ACT tanh (fp16); v=u^2, v2=v^4 by DVE tt; the
               degree-7 polynomial evaluated Estrin-style with 4x-mode fp16
               tensor_scalar ops (per-partition coefficient pairs) and
               2x-mode fp16 tensor_tensor ops; final out = s*g in two fp32
               column halves.
  stores     - kv_writeback descriptors prepared on the SWDGE ring during
               the fit phase; each output half fires via trigger_dma as soon
               as its final multiply lands (no HWDGE issue latency on the
               tail).
"""

import os
import numpy as np

B, J, K = 4, 131072, 512
P, F = 128, 512   # per-core tile (P*F = J/2)
H = F // 2        # output store half-columns
NCORES = 8
NN = 128          # fit nodes (one per partition)
D = 5             # polynomial degree in u
WA = 0.5          # tanh warp: u = tanh(WA*s)
SR = 5.2          # node range: s_n uniform in (-SR, SR)
NE = D // 2 + 1   # even-part coeffs (poly in v=u^2)
NO = (D + 1) // 2 # odd-part coeffs
NC_ = NE + NO

_CONSTS = None
_NC_CACHE = {}


def _host_constants():
    """Data-independent fit constants: node affine map + folded fit matrix."""
    global _CONSTS
    if _CONSTS is not None:
        return _CONSTS
    alpha = 2.0 * SR / NN
    beta = -SR + SR / NN           # s_n = beta + alpha*n,  n = 0..NN-1
    n = np.arange(NN, dtype=np.float64)
    un = np.tanh(WA * (beta + alpha * n))
    vn = un * un
    Vb = np.concatenate([
        np.stack([vn**m for m in range(NE)], 1),
        np.stack([un * vn**m for m in range(NO)], 1)], 1)   # (NN, NC_)
    G = np.linalg.pinv(Vb)                                   # (NC_, NN)
    # gate = 0.5*tanh(0.5*ctx) + 0.5; the affine folds into the fit:
    # c = G @ gate = (0.5*G) @ tanhvals + G @ (0.5*ones)
    gt2 = np.ascontiguousarray((0.5 * G).T.astype(np.float16))   # (NN, NC_)
    coff = np.ascontiguousarray(
        (G @ (0.5 * np.ones(NN))).astype(np.float16).reshape(1, NC_))
    _CONSTS = (float(alpha), float(beta), gt2, coff)
    return _CONSTS


def _build_nc(variant):
    import concourse.bacc as bacc
    import concourse.bass as bass_mod
    import concourse.mybir as mybir
    from concourse import tile

    fp32 = mybir.dt.float32
    fp16 = mybir.dt.float16
    i32 = mybir.dt.int32
    AF = mybir.ActivationFunctionType
    OP = mybir.AluOpType
    alpha, beta, _gt2, _coff = _host_constants()

    nc = bacc.Bacc("TRN2", target_bir_lowering=False, debug=False,
                   num_devices=NCORES, num_swdge_queues=4)
    s_d = nc.dram_tensor("s", (P, F), fp32, kind="ExternalInput")
    qp_d = nc.dram_tensor("qpair", (2, K), fp16, kind="ExternalInput")
    gt_d = nc.dram_tensor("gt2", (NN, NC_), fp16, kind="ExternalInput")
    co_d = nc.dram_tensor("coff", (1, NC_), fp16, kind="ExternalInput")
    y_d = nc.dram_tensor("y", (P, F), fp32, kind="ExternalOutput")

    sem0 = nc.alloc_semaphore("wb0")
    sem1 = nc.alloc_semaphore("wb1")
    sem2 = nc.alloc_semaphore("wb2")

    with tile.TileContext(nc) as tc:
        with (
            tc.tile_pool(name="c1", bufs=1) as cp,
            tc.tile_pool(name="ps", bufs=2, space="PSUM") as pp,
        ):
            # ---------------- Pool (gpsimd) queue: loads + metadata --------
            # bulk s tile via SWDGE (Pool prologue delay hides under the
            # longer q->En critical path); q goes first on SP-HWDGE
            s_all = cp.tile([P, F], fp32, tag="s_all")
            nc.gpsimd.dma_start(out=s_all[:], in_=s_d[:])
            zz = cp.tile([1, 1], fp32, tag="zz")
            nc.gpsimd.memset(zz[:], 0.0)
            io = cp.tile([P, 1], fp32, tag="io")
            nc.gpsimd.iota(io[:], [[1, 1]], channel_multiplier=1,
                           allow_small_or_imprecise_dtypes=True)
            ones2 = cp.tile([2, P], fp16, tag="ones2")
            nc.gpsimd.memset(ones2[:], 1.0)
            ones1 = cp.tile([1, P], fp16, tag="ones1")
            nc.gpsimd.memset(ones1[:], 1.0)
            cidx = cp.tile([P, 1], i32, tag="cidx")
            nc.gpsimd.memset(cidx[:], 0)

            # ---------------- SP queue: q + fit-matrix loads ---------------
            qp_sb = cp.tile([2, K], fp16, tag="qp_sb")
            nc.sync.dma_start(out=qp_sb[:], in_=qp_d[:])
            gtt = cp.tile([NN, NC_], fp16, tag="gtt")
            nc.sync.dma_start(out=gtt[:], in_=gt_d[:])
            coft = cp.tile([1, NC_], fp16, tag="coft")
            nc.sync.dma_start(out=coft[:], in_=co_d[:])

            # output tile + store descriptor prep (SWDGE ring, data deferred)
            outt = cp.tile([P, F], fp32, tag="outt")

            def wb(prep_q, sem, col0, w):
                oh = outt[:, col0:col0 + w]
                in4 = bass_mod.AP(oh.tensor, oh.offset,
                                  [list(oh.ap[0]), [w, 1], [w, 1],
                                   list(oh.ap[-1])])
                ya = y_d[:]
                out4 = bass_mod.AP(ya.tensor, ya.offset + col0,
                                   [[P * F, 1], [F, P], [F, 1], [1, w]])
                return nc.gpsimd.kv_writeback(
                    out4, in4, cidx[:],
                    prepare_only=True, sem=sem, queue_num=prep_q)

            # ---------------- ACT warmup: hoist the act-table load --------
            zz2 = cp.tile([1, 1], fp32, tag="zz2")
            nc.scalar.activation(zz2[:], zz[:], AF.Exp)

            # ---------------- fit-node pipeline ---------------------------
            snt = cp.tile([P, 1], fp32, tag="snt")
            nc.vector.tensor_scalar(out=snt[:], in0=io[:], scalar1=alpha,
                                    scalar2=beta, op0=OP.mult, op1=OP.add)

            q_ps = pp.tile([P, K], fp32, tag="qps")
            nc.tensor.matmul(q_ps[:], ones2[:], qp_sb[:], start=True, stop=True)

            En = cp.tile([NN, K], fp32, tag="En")
            S0n = cp.tile([NN, 1], fp32, tag="S0n")
            nc.scalar.activation(En[:], q_ps[:], AF.Exp, scale=snt[:],
                                 accum_out=S0n[:])
            # warp for the main tile; the zero bias rides on En so the ACT
            # queue schedules the node exp (coeff critical path) first
            zer = cp.tile([P, 1], fp32, tag="zer")
            nc.vector.tensor_scalar(out=zer[:], in0=En[:, 0:1], scalar1=0.0,
                                    scalar2=None, op0=OP.mult)
            T = cp.tile([P, F], fp16, tag="T")
            nc.scalar.activation(T[:], s_all[:], AF.Tanh, scale=float(WA),
                                 bias=zer[:])

            scr = cp.tile([NN, K], fp32, tag="scr")
            S1n = cp.tile([NN, 1], fp32, tag="S1n")
            nc.vector.scalar_tensor_tensor(
                out=scr[:], in0=En[:], scalar=0.5, in1=q_ps[:],
                op0=OP.mult, op1=OP.mult, accum_out=S1n[:])
            recn = cp.tile([NN, 1], fp32, tag="recn")
            nc.vector.reciprocal(recn[:], S0n[:])
            # thn = tanh(0.5*S1/S0) with the 0.5 folded into S1; fp16 so it
            # feeds the PE contraction directly via a stride-0 broadcast AP
            thn = cp.tile([NN, 1], fp16, tag="thn")
            nc.scalar.activation(thn[:], S1n[:], AF.Tanh, scale=recn[:])
            c_ps = pp.tile([P, NC_], fp32, tag="cps")
            nc.tensor.matmul(c_ps[:], thn[:].broadcast_to([NN, P]), gtt[:],
                             start=True, stop=False)
            nc.tensor.matmul(c_ps[:], ones1[:], coft[:], start=False, stop=True)

            # ---------------- main evaluation (DVE, fp16) -----------------
            v = cp.tile([P, F], fp16, tag="v")
            nc.vector.tensor_tensor(v[:], T[:], T[:], OP.mult)
            v2 = cp.tile([P, F], fp16, tag="v2")
            nc.vector.tensor_tensor(v2[:], v[:], v[:], OP.mult)

            c_sb = cp.tile([P, NC_], fp32, tag="csb")
            nc.vector.tensor_copy(c_sb[:], c_ps[:])

            def col(i):
                return c_sb[:, i:i + 1]

            # [a|cc] and [e2t|o2t] packed in wide tiles so the E/O adds run
            # as one 1024-wide 2x tensor_tensor
            ac = cp.tile([P, 2 * F], fp16, tag="ac")
            nc.vector.tensor_scalar(out=ac[:, 0:F], in0=v[:], scalar1=col(1),
                                    scalar2=col(0), op0=OP.mult, op1=OP.add)
            nc.vector.tensor_scalar(out=ac[:, F:2 * F], in0=v[:],
                                    scalar1=col(NE + 1), scalar2=col(NE),
                                    op0=OP.mult, op1=OP.add)
            eo = cp.tile([P, 2 * F], fp16, tag="eo")
            nc.vector.tensor_scalar(out=eo[:, 0:F], in0=v2[:], scalar1=col(2),
                                    scalar2=None, op0=OP.mult)
            nc.vector.tensor_scalar(out=eo[:, F:2 * F], in0=v2[:],
                                    scalar1=col(NE + 2), scalar2=None,
                                    op0=OP.mult)
            EO = cp.tile([P, 2 * F], fp16, tag="EO")
            nc.vector.tensor_tensor(EO[:], eo[:], ac[:], OP.add)
            E, O = EO[:, 0:F], EO[:, F:2 * F]
            t4 = cp.tile([P, F], fp16, tag="t4")
            nc.vector.tensor_tensor(t4[:], T[:], O, OP.mult)
            g = cp.tile([P, F], fp16, tag="g")
            nc.vector.tensor_tensor(g[:], E, t4[:], OP.add)

            # out = s*g in fp32, split 384/128 between DVE and the idle Pool
            # engine; each part fires its store via a pre-prepared SWDGE
            # descriptor (the prep's data read defers to the trigger, so the
            # preps run early on the idle ring once _patch_store_sync moves
            # their data waits onto the triggers)
            W0 = 384
            nc.vector.tensor_tensor(outt[:, 0:W0], s_all[:, 0:W0], g[:, 0:W0],
                                    OP.mult)
            nc.gpsimd.tensor_tensor(outt[:, W0:F], s_all[:, W0:F], g[:, W0:F],
                                    OP.mult)
            wb(1, sem0, 0, 256)
            wb(2, sem1, 256, 128)
            wb(3, sem2, W0, 128)
            nc.gpsimd.trigger_dma(count=None, queue_num=1)
            nc.gpsimd.trigger_dma(count=None, queue_num=2)
            nc.gpsimd.trigger_dma(count=None, queue_num=3)
            nc.gpsimd.wait_ge(sem0, 16)
            nc.gpsimd.wait_ge(sem1, 16)
            nc.gpsimd.wait_ge(sem2, 16)

    _patch_store_sync(nc, mybir)
    nc.compile()
    return nc


def _patch_store_sync(nc, mybir):
    """Post-schedule sync fixups for the triggered SWDGE stores.

    Tile places each writeback prep's (deferred) data wait BEFORE the prep,
    serializing the ~1us descriptor generation behind the final compute.  The
    prep only writes ring descriptors — its source read happens at trigger
    time — so the data wait belongs on the trigger.  Move it there.

    Tile's end-of-kernel drain also waits on the DMASW lane sems it assigned
    to the preps, but a prepare_only descriptor carries the caller's sem
    (wb0/wb1, which we wait on explicitly), so those lane sems never move.
    Strip waits on semaphores that no instruction updates.
    """
    fn = nc.m.functions[0]
    insts = [i for b in fn.blocks for i in b.instructions]
    pool = [i for i in insts if i.engine == mybir.EngineType.Pool]

    def waits(i):
        return list(i.sync_info.on_wait) if i.sync_info else []

    def ups(i):
        return list(i.sync_info.on_update) if i.sync_info else []

    def set_sync(i, w, u):
        i.sync_info = mybir.SyncInfo(on_wait=w, on_update=u)

    # move compute->prep waits onto the same-queue trigger
    for idx, ins in enumerate(pool):
        if type(ins).__name__ != "InstKVWritebackAnt":
            continue
        trig = next(t for t in pool[idx + 1:]
                    if type(t).__name__ == "InstTriggerDma"
                    and t.queue_num == ins.queue_num)
        moved = []
        for src in (pool[idx - 1], ins):
            if src is ins or type(src).__name__ == "InstEventSemaphore":
                keep = []
                for w in waits(src):
                    (moved if (w.ant_name or "").startswith("DVE")
                     else keep).append(w)
                if moved or keep != waits(src):
                    set_sync(src, keep, ups(src))
        if moved:
            set_sync(trig, waits(trig) + moved, ups(trig))

    # strip waits on semaphores nothing updates (dead lane sems)
    updated = {u.id for i in insts for u in ups(i)}
    for ins in insts:
        w = waits(ins)
        keep = [x for x in w if x.id in updated]
        if len(keep) != len(w):
            set_sync(ins, keep, ups(ins))


def _get_nc(variant="fast"):
    if variant not in _NC_CACHE:
        _NC_CACHE[variant] = _build_nc(variant)
    return _NC_CACHE[variant]


def _in_maps(key, query):
    _alpha, _beta, gt2, coff = _host_constants()
    s2 = key.reshape(B, J)
    h = J // 2
    maps = []
    for c in range(NCORES):
        b, half = divmod(c, 2)
        q = query[b].astype(np.float32)
        qhi = q.astype(np.float16)
        qlo = (q - qhi.astype(np.float32)).astype(np.float16)
        maps.append({
            "s": np.ascontiguousarray(
                s2[b, half * h:(half + 1) * h].reshape(P, F)),
            "qpair": np.ascontiguousarray(np.stack([qhi, qlo], 0)),
            "gt2": gt2,
            "coff": coff,
        })
    return maps


def kernel(key, query, _variant=None, _trace=False):
    key = np.ascontiguousarray(key, dtype=np.float32)
    query = np.ascontiguousarray(query, dtype=np.float32)
    nc = _get_nc(_variant or "fast")
    from concourse.bass_utils import run_bass_kernel_spmd

    res = run_bass_kernel_spmd(
        nc, _in_maps(key, query), list(range(NCORES)), trace=_trace)
    h = J // 2
    out = np.empty((B, J), np.float32)
    for c in range(NCORES):
        b, half = divmod(c, 2)
        out[b, half * h:(half + 1) * h] = res.results[c]["y"].reshape(h)
    if _trace:
        kernel.last_results = res
    return out.reshape(key.shape)


# revision 20
# speedup vs baseline: 1.2675x; 1.2675x over previous
"""Trainium2 Bass kernel for nn_AttentionMask_13048110645633.

Math: for key (4,32,64,64) and query (4,512), with s = key.reshape(B,J) and
q = query, the reference computes elementwise

    ctx[b,j] = sum_k q[b,k]*exp(s[b,j]*q[b,k]) / sum_k exp(s[b,j]*q[b,k])
    out[b,j] = s[b,j] * sigmoid(ctx[b,j])

i.e. out = s * g_b(s) with g_b a smooth scalar gate determined by q[b].
Sharding: data-parallel over B (4 batches x 2 half-slabs = 8 cores), one
(128,512) tile per core.

Device algorithm (per core):
  fit phase  - 128 fit nodes s_n synthesized on-device (iota+affine, no DMA);
               q broadcast to all partitions by a C=2 fp16-pair PE matmul;
               one ACT exp with per-partition scale + accumulate gives
               En=exp(s_n q) and S0; one DVE stt multiply-reduce gives
               0.5*S1; tanh(S1/(2*S0)) on ACT; the sigmoid affine and the
               least-squares fit both fold into a host fit matrix applied by
               two tiny PE matmuls -> even/odd polynomial coefficients of
               g in u = tanh(0.4 s), replicated on all 128 partitions.
  eval phase - u from one ACT tanh (fp16); v=u^2, v2=v^4 by DVE tt; the
               degree-7 polynomial evaluated Estrin-style with 4x-mode fp16
               tensor_scalar ops (per-partition coefficient pairs) and
               2x-mode fp16 tensor_tensor ops; final out = s*g in two fp32
               column halves.
  stores     - kv_writeback descriptors prepared on the SWDGE ring during
               the fit phase; each output half fires via trigger_dma as soon
               as its final multiply lands (no HWDGE issue latency on the
               tail).
"""

import os
import numpy as np

B, J, K = 4, 131072, 512
P, F = 128, 512   # per-core tile (P*F = J/2)
H = F // 2        # output store half-columns
NCORES = 8
NN = 128          # fit nodes (one per partition)
D = 5             # polynomial degree in u
WA = 0.5          # tanh warp: u = tanh(WA*s)
SR = 5.2          # node range: s_n uniform in (-SR, SR)
NE = D // 2 + 1   # even-part coeffs (poly in v=u^2)
NO = (D + 1) // 2 # odd-part coeffs
NC_ = NE + NO

_CONSTS = None
_NC_CACHE = {}


def _host_constants():
    """Data-independent fit constants: node affine map + folded fit matrix."""
    global _CONSTS
    if _CONSTS is not None:
        return _CONSTS
    alpha = 2.0 * SR / NN
    beta = -SR + SR / NN           # s_n = beta + alpha*n,  n = 0..NN-1
    n = np.arange(NN, dtype=np.float64)
    un = np.tanh(WA * (beta + alpha * n))
    vn = un * un
    Vb = np.concatenate([
        np.stack([vn**m for m in range(NE)], 1),
        np.stack([un * vn**m for m in range(NO)], 1)], 1)   # (NN, NC_)
    G = np.linalg.pinv(Vb)                                   # (NC_, NN)
    # gate = 0.5*tanh(0.5*ctx) + 0.5; the affine folds into the fit:
    # c = G @ gate = (0.5*G) @ tanhvals + G @ (0.5*ones)
    gt2 = np.ascontiguousarray((0.5 * G).T.astype(np.float16))   # (NN, NC_)
    coff = np.ascontiguousarray(
        (G @ (0.5 * np.ones(NN))).astype(np.float16).reshape(1, NC_))
    _CONSTS = (float(alpha), float(beta), gt2, coff)
    return _CONSTS


def _build_nc(variant):
    import concourse.bacc as bacc
    import concourse.bass as bass_mod
    import concourse.mybir as mybir
    from concourse import tile

    fp32 = mybir.dt.float32
    fp16 = mybir.dt.float16
    i32 = mybir.dt.int32
    AF = mybir.ActivationFunctionType
    OP = mybir.AluOpType
    alpha, beta, _gt2, _coff = _host_constants()

    nc = bacc.Bacc("TRN2", target_bir_lowering=False, debug=False,
                   num_devices=NCORES, num_swdge_queues=2)
    s_d = nc.dram_tensor("s", (P, F), fp32, kind="ExternalInput")
    qp_d = nc.dram_tensor("qpair", (2, K), fp16, kind="ExternalInput")
    gt_d = nc.dram_tensor("gt2", (NN, NC_), fp16, kind="ExternalInput")
    co_d = nc.dram_tensor("coff", (1, NC_), fp16, kind="ExternalInput")
    y_d = nc.dram_tensor("y", (P, F), fp32, kind="ExternalOutput")

    sem0 = nc.alloc_semaphore("wb0")

    with tile.TileContext(nc) as tc:
        with (
            tc.tile_pool(name="c1", bufs=1) as cp,
            tc.tile_pool(name="ps", bufs=2, space="PSUM") as pp,
        ):
            # ---------------- Pool (gpsimd) queue: loads + metadata --------
            # bulk s tile via SWDGE (Pool prologue delay hides under the
            # longer q->En critical path); q goes first on SP-HWDGE
            s_all = cp.tile([P, F], fp32, tag="s_all")
            nc.gpsimd.dma_start(out=s_all[:], in_=s_d[:])
            zz = cp.tile([1, 1], fp32, tag="zz")
            nc.gpsimd.memset(zz[:], 0.0)
            io = cp.tile([P, 1], fp32, tag="io")
            nc.gpsimd.iota(io[:], [[1, 1]], channel_multiplier=1,
                           allow_small_or_imprecise_dtypes=True)
            ones2 = cp.tile([2, P], fp16, tag="ones2")
            nc.gpsimd.memset(ones2[:], 1.0)
            ones1 = cp.tile([1, P], fp16, tag="ones1")
            nc.gpsimd.memset(ones1[:], 1.0)
            cidx = cp.tile([P, 1], i32, tag="cidx")
            nc.gpsimd.memset(cidx[:], 0)

            # ---------------- SP queue: q + fit-matrix loads ---------------
            qp_sb = cp.tile([2, K], fp16, tag="qp_sb")
            nc.sync.dma_start(out=qp_sb[:], in_=qp_d[:])
            gtt = cp.tile([NN, NC_], fp16, tag="gtt")
            nc.sync.dma_start(out=gtt[:], in_=gt_d[:])
            coft = cp.tile([1, NC_], fp16, tag="coft")
            nc.sync.dma_start(out=coft[:], in_=co_d[:])

            # output tile + store descriptor prep (SWDGE ring, data deferred)
            outt = cp.tile([P, F], fp32, tag="outt")

            def wb(prep_q, sem, col0, w):
                oh = outt[:, col0:col0 + w]
                in4 = bass_mod.AP(oh.tensor, oh.offset,
                                  [list(oh.ap[0]), [w, 1], [w, 1],
                                   list(oh.ap[-1])])
                ya = y_d[:]
                out4 = bass_mod.AP(ya.tensor, ya.offset + col0,
                                   [[P * F, 1], [F, P], [F, 1], [1, w]])
                return nc.gpsimd.kv_writeback(
                    out4, in4, cidx[:],
                    prepare_only=True, sem=sem, queue_num=prep_q)

            # ---------------- ACT warmup: hoist the act-table load --------
            zz2 = cp.tile([1, 1], fp32, tag="zz2")
            nc.scalar.activation(zz2[:], zz[:], AF.Exp)

            # ---------------- fit-node pipeline ---------------------------
            snt = cp.tile([P, 1], fp32, tag="snt")
            nc.vector.tensor_scalar(out=snt[:], in0=io[:], scalar1=alpha,
                                    scalar2=beta, op0=OP.mult, op1=OP.add)

            q_ps = pp.tile([P, K], fp32, tag="qps")
            nc.tensor.matmul(q_ps[:], ones2[:], qp_sb[:], start=True, stop=True)

            En = cp.tile([NN, K], fp32, tag="En")
            S0n = cp.tile([NN, 1], fp32, tag="S0n")
            nc.scalar.activation(En[:], q_ps[:], AF.Exp, scale=snt[:],
                                 accum_out=S0n[:])
            # warp for the main tile; the zero bias rides on En so the ACT
            # queue schedules the node exp (coeff critical path) first
            zer = cp.tile([P, 1], fp32, tag="zer")
            nc.vector.tensor_scalar(out=zer[:], in0=En[:, 0:1], scalar1=0.0,
                                    scalar2=None, op0=OP.mult)
            T = cp.tile([P, F], fp16, tag="T")
            nc.scalar.activation(T[:], s_all[:], AF.Tanh, scale=float(WA),
                                 bias=zer[:])

            scr = cp.tile([NN, K], fp32, tag="scr")
            S1n = cp.tile([NN, 1], fp32, tag="S1n")
            nc.vector.scalar_tensor_tensor(
                out=scr[:], in0=En[:], scalar=0.5, in1=q_ps[:],
                op0=OP.mult, op1=OP.mult, accum_out=S1n[:])
            recn = cp.tile([NN, 1], fp32, tag="recn")
            nc.vector.reciprocal(recn[:], S0n[:])
            # thn = tanh(0.5*S1/S0) with the 0.5 folded into S1; fp16 so it
            # feeds the PE contraction directly via a stride-0 broadcast AP
            thn = cp.tile([NN, 1], fp16, tag="thn")
            nc.scalar.activation(thn[:], S1n[:], AF.Tanh, scale=recn[:])
            c_ps = pp.tile([P, NC_], fp32, tag="cps")
            nc.tensor.matmul(c_ps[:], thn[:].broadcast_to([NN, P]), gtt[:],
                             start=True, stop=False)
            nc.tensor.matmul(c_ps[:], ones1[:], coft[:], start=False, stop=True)

            # ---------------- main evaluation (DVE, fp16) -----------------
            v = cp.tile([P, F], fp16, tag="v")
            nc.vector.tensor_tensor(v[:], T[:], T[:], OP.mult)
            v2 = cp.tile([P, F], fp16, tag="v2")
            nc.vector.tensor_tensor(v2[:], v[:], v[:], OP.mult)

            c_sb = cp.tile([P, NC_], fp32, tag="csb")
            nc.vector.tensor_copy(c_sb[:], c_ps[:])

            def col(i):
                return c_sb[:, i:i + 1]

            # [a|cc] and [e2t|o2t] packed in wide tiles so the E/O adds run
            # as one 1024-wide 2x tensor_tensor
            ac = cp.tile([P, 2 * F], fp16, tag="ac")
            nc.vector.tensor_scalar(out=ac[:, 0:F], in0=v[:], scalar1=col(1),
                                    scalar2=col(0), op0=OP.mult, op1=OP.add)
            nc.vector.tensor_scalar(out=ac[:, F:2 * F], in0=v[:],
                                    scalar1=col(NE + 1), scalar2=col(NE),
                                    op0=OP.mult, op1=OP.add)
            eo = cp.tile([P, 2 * F], fp16, tag="eo")
            nc.vector.tensor_scalar(out=eo[:, 0:F], in0=v2[:], scalar1=col(2),
                                    scalar2=None, op0=OP.mult)
            nc.vector.tensor_scalar(out=eo[:, F:2 * F], in0=v2[:],
                                    scalar1=col(NE + 2), scalar2=None,
                                    op0=OP.mult)
            EO = cp.tile([P, 2 * F], fp16, tag="EO")
            nc.vector.tensor_tensor(EO[:], eo[:], ac[:], OP.add)
            E, O = EO[:, 0:F], EO[:, F:2 * F]
            t4 = cp.tile([P, F], fp16, tag="t4")
            nc.vector.tensor_tensor(t4[:], T[:], O, OP.mult)
            g = cp.tile([P, F], fp16, tag="g")
            nc.vector.tensor_tensor(g[:], E, t4[:], OP.add)

            # out = s*g in fp32; the store fires via a pre-prepared SWDGE
            # descriptor (the prep's data read defers to the trigger, so the
            # prep itself runs early on the idle ring once _patch_store_sync
            # moves its data wait onto the trigger)
            nc.vector.tensor_tensor(outt[:], s_all[:], g[:], OP.mult)
            wb(1, sem0, 0, F)
            nc.gpsimd.trigger_dma(count=None, queue_num=1)
            nc.gpsimd.wait_ge(sem0, 16)

    _patch_store_sync(nc, mybir)
    nc.compile()
    return nc


def _patch_store_sync(nc, mybir):
    """Post-schedule sync fixups for the triggered SWDGE stores.

    Tile places each writeback prep's (deferred) data wait BEFORE the prep,
    serializing the ~1us descriptor generation behind the final compute.  The
    prep only writes ring descriptors — its source read happens at trigger
    time — so the data wait belongs on the trigger.  Move it there.

    Tile's end-of-kernel drain also waits on the DMASW lane sems it assigned
    to the preps, but a prepare_only descriptor carries the caller's sem
    (wb0/wb1, which we wait on explicitly), so those lane sems never move.
    Strip waits on semaphores that no instruction updates.
    """
    fn = nc.m.functions[0]
    insts = [i for b in fn.blocks for i in b.instructions]
    pool = [i for i in insts if i.engine == mybir.EngineType.Pool]

    def waits(i):
        return list(i.sync_info.on_wait) if i.sync_info else []

    def ups(i):
        return list(i.sync_info.on_update) if i.sync_info else []

    def set_sync(i, w, u):
        i.sync_info = mybir.SyncInfo(on_wait=w, on_update=u)

    # move compute->prep waits onto the same-queue trigger
    for idx, ins in enumerate(pool):
        if type(ins).__name__ != "InstKVWritebackAnt":
            continue
        trig = next(t for t in pool[idx + 1:]
                    if type(t).__name__ == "InstTriggerDma"
                    and t.queue_num == ins.queue_num)
        moved = []
        for src in (pool[idx - 1], ins):
            if src is ins or type(src).__name__ == "InstEventSemaphore":
                keep = []
                for w in waits(src):
                    (moved if (w.ant_name or "").startswith("DVE")
                     else keep).append(w)
                if moved or keep != waits(src):
                    set_sync(src, keep, ups(src))
        if moved:
            set_sync(trig, waits(trig) + moved, ups(trig))

    # strip waits on semaphores nothing updates (dead lane sems)
    updated = {u.id for i in insts for u in ups(i)}
    for ins in insts:
        w = waits(ins)
        keep = [x for x in w if x.id in updated]
        if len(keep) != len(w):
            set_sync(ins, keep, ups(ins))


def _get_nc(variant="fast"):
    if variant not in _NC_CACHE:
        _NC_CACHE[variant] = _build_nc(variant)
    return _NC_CACHE[variant]


def _in_maps(key, query):
    _alpha, _beta, gt2, coff = _host_constants()
    s2 = key.reshape(B, J)
    h = J // 2
    maps = []
    for c in range(NCORES):
        b, half = divmod(c, 2)
        q = query[b].astype(np.float32)
        qhi = q.astype(np.float16)
        qlo = (q - qhi.astype(np.float32)).astype(np.float16)
        maps.append({
            "s": np.ascontiguousarray(
                s2[b, half * h:(half + 1) * h].reshape(P, F)),
            "qpair": np.ascontiguousarray(np.stack([qhi, qlo], 0)),
            "gt2": gt2,
            "coff": coff,
        })
    return maps


def kernel(key, query, _variant=None, _trace=False):
    key = np.ascontiguousarray(key, dtype=np.float32)
    query = np.ascontiguousarray(query, dtype=np.float32)
    nc = _get_nc(_variant or "fast")
    from concourse.bass_utils import run_bass_kernel_spmd

    res = run_bass_kernel_spmd(
        nc, _in_maps(key, query), list(range(NCORES)), trace=_trace)
    h = J // 2
    out = np.empty((B, J), np.float32)
    for c in range(NCORES):
        b, half = divmod(c, 2)
        out[b, half * h:(half + 1) * h] = res.results[c]["y"].reshape(h)
    if _trace:
        kernel.last_results = res
    return out.reshape(key.shape)


# revision 21
# speedup vs baseline: 1.2719x; 1.0034x over previous
"""Trainium2 Bass kernel for nn_AttentionMask_13048110645633.

Math: for key (4,32,64,64) and query (4,512), with s = key.reshape(B,J) and
q = query, the reference computes elementwise

    ctx[b,j] = sum_k q[b,k]*exp(s[b,j]*q[b,k]) / sum_k exp(s[b,j]*q[b,k])
    out[b,j] = s[b,j] * sigmoid(ctx[b,j])

i.e. out = s * g_b(s) with g_b a smooth scalar gate determined by q[b].
Sharding: data-parallel over B (4 batches x 2 half-slabs = 8 cores), one
(128,512) tile per core.

Device algorithm (per core):
  fit phase  - 128 fit nodes s_n synthesized on-device (iota+affine, no DMA);
               q broadcast to all partitions by a C=2 fp16-pair PE matmul;
               one ACT exp with per-partition scale + accumulate gives
               En=exp(s_n q) and S0; one DVE stt multiply-reduce gives
               0.5*S1; tanh(S1/(2*S0)) on ACT; the sigmoid affine and the
               least-squares fit both fold into a host fit matrix applied by
               two tiny PE matmuls -> even/odd polynomial coefficients of
               g in u = tanh(0.4 s), replicated on all 128 partitions.
  eval phase - u from one ACT tanh (fp16); v=u^2, v2=v^4 by DVE tt; the
               degree-7 polynomial evaluated Estrin-style with 4x-mode fp16
               tensor_scalar ops (per-partition coefficient pairs) and
               2x-mode fp16 tensor_tensor ops; final out = s*g in two fp32
               column halves.
  stores     - kv_writeback descriptors prepared on the SWDGE ring during
               the fit phase; each output half fires via trigger_dma as soon
               as its final multiply lands (no HWDGE issue latency on the
               tail).
"""

import os
import numpy as np

B, J, K = 4, 131072, 512
P, F = 128, 512   # per-core tile (P*F = J/2)
H = F // 2        # output store half-columns
NCORES = 8
NN = 128          # fit nodes (one per partition)
D = 5             # polynomial degree in u
WA = 0.5          # tanh warp: u = tanh(WA*s)
SR = 5.2          # node range: s_n uniform in (-SR, SR)
NE = D // 2 + 1   # even-part coeffs (poly in v=u^2)
NO = (D + 1) // 2 # odd-part coeffs
NC_ = NE + NO

_CONSTS = None
_NC_CACHE = {}


def _host_constants():
    """Data-independent fit constants: node affine map + folded fit matrix."""
    global _CONSTS
    if _CONSTS is not None:
        return _CONSTS
    alpha = 2.0 * SR / NN
    beta = -SR + SR / NN           # s_n = beta + alpha*n,  n = 0..NN-1
    n = np.arange(NN, dtype=np.float64)
    un = np.tanh(WA * (beta + alpha * n))
    vn = un * un
    Vb = np.concatenate([
        np.stack([vn**m for m in range(NE)], 1),
        np.stack([un * vn**m for m in range(NO)], 1)], 1)   # (NN, NC_)
    G = np.linalg.pinv(Vb)                                   # (NC_, NN)
    # gate = 0.5*tanh(0.5*ctx) + 0.5; the affine folds into the fit:
    # c = G @ gate = (0.5*G) @ tanhvals + G @ (0.5*ones)
    gt2 = np.ascontiguousarray((0.5 * G).T.astype(np.float16))   # (NN, NC_)
    coff = np.ascontiguousarray(
        (G @ (0.5 * np.ones(NN))).astype(np.float16).reshape(1, NC_))
    _CONSTS = (float(alpha), float(beta), gt2, coff)
    return _CONSTS


def _build_nc(variant):
    import concourse.bacc as bacc
    import concourse.bass as bass_mod
    import concourse.mybir as mybir
    from concourse import tile

    fp32 = mybir.dt.float32
    fp16 = mybir.dt.float16
    i32 = mybir.dt.int32
    AF = mybir.ActivationFunctionType
    OP = mybir.AluOpType
    alpha, beta, _gt2, _coff = _host_constants()

    nc = bacc.Bacc("TRN2", target_bir_lowering=False, debug=False,
                   num_devices=NCORES, num_swdge_queues=2)
    s_d = nc.dram_tensor("s", (P, F), fp32, kind="ExternalInput")
    qp_d = nc.dram_tensor("qpair", (2, K), fp16, kind="ExternalInput")
    gt_d = nc.dram_tensor("gt2", (NN, NC_), fp16, kind="ExternalInput")
    co_d = nc.dram_tensor("coff", (1, NC_), fp16, kind="ExternalInput")
    y_d = nc.dram_tensor("y", (P, F), fp32, kind="ExternalOutput")

    sem0 = nc.alloc_semaphore("wb0")

    with tile.TileContext(nc) as tc:
        with (
            tc.tile_pool(name="c1", bufs=1) as cp,
            tc.tile_pool(name="ps", bufs=2, space="PSUM") as pp,
        ):
            # ---------------- Pool (gpsimd) queue: loads + metadata --------
            # bulk s tile via SWDGE (Pool prologue delay hides under the
            # longer q->En critical path); q goes first on SP-HWDGE
            s_all = cp.tile([P, F], fp32, tag="s_all")
            nc.gpsimd.dma_start(out=s_all[:], in_=s_d[:])
            zz = cp.tile([1, 1], fp32, tag="zz")
            nc.gpsimd.memset(zz[:], 0.0)
            io = cp.tile([P, 1], fp32, tag="io")
            nc.gpsimd.iota(io[:], [[1, 1]], channel_multiplier=1,
                           allow_small_or_imprecise_dtypes=True)
            ones2 = cp.tile([2, P], fp16, tag="ones2")
            nc.gpsimd.memset(ones2[:], 1.0)
            ones1 = cp.tile([1, P], fp16, tag="ones1")
            nc.gpsimd.memset(ones1[:], 1.0)
            cidx = cp.tile([P, 1], i32, tag="cidx")
            nc.gpsimd.memset(cidx[:], 0)

            # ---------------- SP queue: q + fit-matrix loads ---------------
            qp_sb = cp.tile([2, K], fp16, tag="qp_sb")
            nc.sync.dma_start(out=qp_sb[:], in_=qp_d[:])
            gtt = cp.tile([NN, NC_], fp16, tag="gtt")
            nc.sync.dma_start(out=gtt[:], in_=gt_d[:])
            coft = cp.tile([1, NC_], fp16, tag="coft")
            nc.sync.dma_start(out=coft[:], in_=co_d[:])

            # output tile + store descriptor prep (SWDGE ring, data deferred)
            outt = cp.tile([P, F], fp32, tag="outt")

            def wb(prep_q, sem, col0, w):
                oh = outt[:, col0:col0 + w]
                in4 = bass_mod.AP(oh.tensor, oh.offset,
                                  [list(oh.ap[0]), [w, 1], [w, 1],
                                   list(oh.ap[-1])])
                ya = y_d[:]
                out4 = bass_mod.AP(ya.tensor, ya.offset + col0,
                                   [[P * F, 1], [F, P], [F, 1], [1, w]])
                return nc.gpsimd.kv_writeback(
                    out4, in4, cidx[:],
                    prepare_only=True, sem=sem, queue_num=prep_q)

            # ---------------- ACT warmup: hoist the act-table load --------
            zz2 = cp.tile([1, 1], fp32, tag="zz2")
            nc.scalar.activation(zz2[:], zz[:], AF.Exp)

            # ---------------- fit-node pipeline ---------------------------
            snt = cp.tile([P, 1], fp32, tag="snt")
            nc.vector.tensor_scalar(out=snt[:], in0=io[:], scalar1=alpha,
                                    scalar2=beta, op0=OP.mult, op1=OP.add)

            q_ps = pp.tile([P, K], fp32, tag="qps")
            nc.tensor.matmul(q_ps[:], ones2[:], qp_sb[:], start=True, stop=True)

            En = cp.tile([NN, K], fp32, tag="En")
            S0n = cp.tile([NN, 1], fp32, tag="S0n")
            nc.scalar.activation(En[:], q_ps[:], AF.Exp, scale=snt[:],
                                 accum_out=S0n[:])
            # warp for the main tile; the zero bias rides on En so the ACT
            # queue schedules the node exp (coeff critical path) first
            zer = cp.tile([P, 1], fp32, tag="zer")
            nc.vector.tensor_scalar(out=zer[:], in0=En[:, 0:1], scalar1=0.0,
                                    scalar2=None, op0=OP.mult)
            T = cp.tile([P, F], fp16, tag="T")
            nc.scalar.activation(T[:], s_all[:], AF.Tanh, scale=float(WA),
                                 bias=zer[:])

            scr = cp.tile([NN, K], fp32, tag="scr")
            S1n = cp.tile([NN, 1], fp32, tag="S1n")
            nc.vector.scalar_tensor_tensor(
                out=scr[:], in0=En[:], scalar=0.5, in1=q_ps[:],
                op0=OP.mult, op1=OP.mult, accum_out=S1n[:])
            recn = cp.tile([NN, 1], fp32, tag="recn")
            nc.vector.reciprocal(recn[:], S0n[:])
            # thn = tanh(0.5*S1/S0) with the 0.5 folded into S1; fp16 so it
            # feeds the PE contraction directly via a stride-0 broadcast AP
            thn = cp.tile([NN, 1], fp16, tag="thn")
            nc.scalar.activation(thn[:], S1n[:], AF.Tanh, scale=recn[:])
            c_ps = pp.tile([P, NC_], fp32, tag="cps")
            nc.tensor.matmul(c_ps[:], thn[:].broadcast_to([NN, P]), gtt[:],
                             start=True, stop=False)
            nc.tensor.matmul(c_ps[:], ones1[:], coft[:], start=False, stop=True)

            # ---------------- main evaluation (DVE, fp16) -----------------
            v = cp.tile([P, F], fp16, tag="v")
            nc.vector.tensor_tensor(v[:], T[:], T[:], OP.mult)
            v2 = cp.tile([P, F], fp16, tag="v2")

            # coefficients to SBUF on the idle ACT engine; the v-dependent
            # ts pair runs before v2 so the chain starts one op earlier
            c_sb = cp.tile([P, NC_], fp32, tag="csb")
            nc.scalar.activation(c_sb[:], c_ps[:], AF.Copy)

            def col(i):
                return c_sb[:, i:i + 1]

            # [a|cc] and [e2t|o2t] packed in wide tiles so the E/O adds run
            # as one 1024-wide 2x tensor_tensor
            ac = cp.tile([P, 2 * F], fp16, tag="ac")
            nc.vector.tensor_scalar(out=ac[:, 0:F], in0=v[:], scalar1=col(1),
                                    scalar2=col(0), op0=OP.mult, op1=OP.add)
            nc.vector.tensor_scalar(out=ac[:, F:2 * F], in0=v[:],
                                    scalar1=col(NE + 1), scalar2=col(NE),
                                    op0=OP.mult, op1=OP.add)
            nc.vector.tensor_tensor(v2[:], v[:], v[:], OP.mult)
            eo = cp.tile([P, 2 * F], fp16, tag="eo")
            nc.vector.tensor_scalar(out=eo[:, 0:F], in0=v2[:], scalar1=col(2),
                                    scalar2=None, op0=OP.mult)
            nc.vector.tensor_scalar(out=eo[:, F:2 * F], in0=v2[:],
                                    scalar1=col(NE + 2), scalar2=None,
                                    op0=OP.mult)
            EO = cp.tile([P, 2 * F], fp16, tag="EO")
            nc.vector.tensor_tensor(EO[:], eo[:], ac[:], OP.add)
            E, O = EO[:, 0:F], EO[:, F:2 * F]
            t4 = cp.tile([P, F], fp16, tag="t4")
            nc.vector.tensor_tensor(t4[:], T[:], O, OP.mult)
            g = cp.tile([P, F], fp16, tag="g")
            nc.vector.tensor_tensor(g[:], E, t4[:], OP.add)

            # out = s*g in fp32; the store fires via a pre-prepared SWDGE
            # descriptor (the prep's data read defers to the trigger, so the
            # prep itself runs early on the idle ring once _patch_store_sync
            # moves its data wait onto the trigger)
            nc.vector.tensor_tensor(outt[:], s_all[:], g[:], OP.mult)
            wb(1, sem0, 0, F)
            nc.gpsimd.trigger_dma(count=None, queue_num=1)
            nc.gpsimd.wait_ge(sem0, 16)

    _patch_store_sync(nc, mybir)
    nc.compile()
    return nc


def _patch_store_sync(nc, mybir):
    """Post-schedule sync fixups for the triggered SWDGE stores.

    Tile places each writeback prep's (deferred) data wait BEFORE the prep,
    serializing the ~1us descriptor generation behind the final compute.  The
    prep only writes ring descriptors — its source read happens at trigger
    time — so the data wait belongs on the trigger.  Move it there.

    Tile's end-of-kernel drain also waits on the DMASW lane sems it assigned
    to the preps, but a prepare_only descriptor carries the caller's sem
    (wb0/wb1, which we wait on explicitly), so those lane sems never move.
    Strip waits on semaphores that no instruction updates.
    """
    fn = nc.m.functions[0]
    insts = [i for b in fn.blocks for i in b.instructions]
    pool = [i for i in insts if i.engine == mybir.EngineType.Pool]

    def waits(i):
        return list(i.sync_info.on_wait) if i.sync_info else []

    def ups(i):
        return list(i.sync_info.on_update) if i.sync_info else []

    def set_sync(i, w, u):
        i.sync_info = mybir.SyncInfo(on_wait=w, on_update=u)

    # move compute->prep waits onto the same-queue trigger
    for idx, ins in enumerate(pool):
        if type(ins).__name__ != "InstKVWritebackAnt":
            continue
        trig = next(t for t in pool[idx + 1:]
                    if type(t).__name__ == "InstTriggerDma"
                    and t.queue_num == ins.queue_num)
        moved = []
        for src in (pool[idx - 1], ins):
            if src is ins or type(src).__name__ == "InstEventSemaphore":
                keep = []
                for w in waits(src):
                    (moved if (w.ant_name or "").startswith("DVE")
                     else keep).append(w)
                if moved or keep != waits(src):
                    set_sync(src, keep, ups(src))
        if moved:
            set_sync(trig, waits(trig) + moved, ups(trig))

    # strip waits on semaphores nothing updates (dead lane sems)
    updated = {u.id for i in insts for u in ups(i)}
    for ins in insts:
        w = waits(ins)
        keep = [x for x in w if x.id in updated]
        if len(keep) != len(w):
            set_sync(ins, keep, ups(ins))


def _get_nc(variant="fast"):
    if variant not in _NC_CACHE:
        _NC_CACHE[variant] = _build_nc(variant)
    return _NC_CACHE[variant]


def _in_maps(key, query):
    _alpha, _beta, gt2, coff = _host_constants()
    s2 = key.reshape(B, J)
    h = J // 2
    maps = []
    for c in range(NCORES):
        b, half = divmod(c, 2)
        q = query[b].astype(np.float32)
        qhi = q.astype(np.float16)
        qlo = (q - qhi.astype(np.float32)).astype(np.float16)
        maps.append({
            "s": np.ascontiguousarray(
                s2[b, half * h:(half + 1) * h].reshape(P, F)),
            "qpair": np.ascontiguousarray(np.stack([qhi, qlo], 0)),
            "gt2": gt2,
            "coff": coff,
        })
    return maps


def kernel(key, query, _variant=None, _trace=False):
    key = np.ascontiguousarray(key, dtype=np.float32)
    query = np.ascontiguousarray(query, dtype=np.float32)
    nc = _get_nc(_variant or "fast")
    from concourse.bass_utils import run_bass_kernel_spmd

    res = run_bass_kernel_spmd(
        nc, _in_maps(key, query), list(range(NCORES)), trace=_trace)
    h = J // 2
    out = np.empty((B, J), np.float32)
    for c in range(NCORES):
        b, half = divmod(c, 2)
        out[b, half * h:(half + 1) * h] = res.results[c]["y"].reshape(h)
    if _trace:
        kernel.last_results = res
    return out.reshape(key.shape)
